# revision 26
# baseline (speedup 1.0000x reference)
"""Trainium2 Bass kernel for the CNN-MAD per-class DTW transport cost.

Math (reference):
  mat_cost[n, j] = C1[n] + C2[c_n, j] - 2*C3[n, j],  c_n = classes[n]
    C1[n]    = sum_t rowsum[c_n, t] * r[n,t],   r[n,t] = sum_d X[n,t,d]^2
    C2[c, j] = sum_p colsum[c, p]  * q[j,p],    q[j,p] = sum_d Y[j,p,d]^2
    C3[n, j] = sum_{p,d} XW[n,p,d] * Y[j,p,d],  XW = pi_c.T @ X (warp)

Sharding: 4x2 grid. Core (rr, cj) owns the samples of classes {2rr, 2rr+1}
(zero-padded to cap1 rows per class, NL = 2*cap1) and the j-half
[512*cj, 512*(cj+1)).  One SPMD program for all 8 cores; per-core class
structure enters only through data.  Host ships fp8 row-norms q/r and the
tiny pi col/row sums; the device runs all four contractions on the PE:
  - warp XW = piS.T @ X at fp8 DoubleRow rate, psum evac'd as a pure
    contiguous copy ((d,n)-major layout, -2 prefolded into the shipped Y).
  - C3 flipped to [j-partition, n-free] psum orientation: 4 j-blocks of
    128, 8 DR passes each over k=(p,d); cost scales with n=NL not NY.
  - C1/C2 as single DR matmuls against r/q; their rows enter each output
    psum via one rank-3 fp16 augmentation matmul per j-block (issued
    early, as the psum group starter).
  - outputs leave via SWDGE prepare/trigger writebacks (per j-block
    queues) so the post-compute DMA latency is trigger+transfer+sem only.
"""

import sys

sys.path.insert(0, "/opt/trn_rl_repo")

import numpy as np

N, NY, T, TP, D, C = 1024, 1024, 256, 256, 8, 8
NCORES = 8
NYL = 512  # j columns per core

_cache = {}

# engines for the 12 warp-psum evacs (rotation), the 4 out evacs, c1/c2
XW_EVAC = ("a", "d", "p")
OUT_EVAC = ("a", "d", "a", "d")
N_PRIME = 52  # PE p-state priming matmuls (0 = off)
WB_JBS = (0, 1, 2, 3)  # j-blocks whose output goes via prepare/trigger writeback


def _copy(nc, eng, dst, src):
    if eng == "a":
        return nc.scalar.mul(dst, src, 1.0)
    elif eng == "d":
        return nc.vector.tensor_copy(dst, src)
    else:
        return nc.gpsimd.tensor_copy(dst, src)


def _build(cap1):
    import bass_rust as _br
    import concourse.bacc as bacc
    import concourse.mybir as mybir
    import concourse.tile as tile

    f8 = mybir.dt.float8e4
    bf = mybir.dt.bfloat16
    f16 = mybir.dt.float16
    f32 = mybir.dt.float32
    i32 = mybir.dt.int32
    DR = mybir.MatmulPerfMode.DoubleRow
    NL = 2 * cap1

    # pqs column map (fp8): piS | q | r | colsum | rowsum
    QO = 1024          # q offset
    RO = QO + 1024     # r offset
    CO = RO + 2 * NL   # colsum offset [pc, 3]
    WO = CO + 6        # rowsum offset [tc, 2]
    PQS = WO + 4

    nwb = len(WB_JBS)
    nc = bacc.Bacc(
        "TRN2",
        target_bir_lowering=False,
        debug=False,
        num_devices=NCORES,
        num_swdge_queues=max(1, nwb),
    )

    pqs_d = nc.dram_tensor("pqs", [128, PQS], f8, kind="ExternalInput")
    xt2_d = nc.dram_tensor("xt2", [128, 16 * NL], f8, kind="ExternalInput")
    ytl_d = nc.dram_tensor("ytl", [128, 16 * NYL], f8, kind="ExternalInput")
    aux_d = nc.dram_tensor("aux", [4, NYL + NL + 16], f16, kind="ExternalInput")
    out_d = nc.dram_tensor("out", [NYL, NL], bf, kind="ExternalOutput")

    with tile.TileContext(nc) as tc:
        with (
            tc.tile_pool(name="io", bufs=1) as pio,
            tc.tile_pool(name="work", bufs=1) as pw,
            tc.tile_pool(name="small", bufs=1) as psm,
            tc.tile_pool(name="ps", bufs=1, space="PSUM") as pp,
        ):
            pqs = pio.tile([128, PQS], f8, tag="pqs")
            xt2 = pio.tile([128, 16 * NL], f8, tag="xt2")
            ytl = pio.tile([128, 16 * NYL], f8, tag="ytl")
            aux = psm.tile([4, NYL + NL + 16], f16, tag="aux")

            piSv = pqs[:, 0:1024].rearrange("l (c t p) -> l c t p", c=2, t=2)
            qv = pqs[:, QO:RO].rearrange("l (pc j) -> l pc j", pc=2)
            rv = pqs[:, RO:CO].rearrange("l (tc n) -> l tc n", tc=2)
            csv = pqs[:, CO:WO].rearrange("l (pc c) -> l pc c", pc=2)
            rsv = pqs[:, WO : WO + 4].rearrange("l (tc c) -> l tc c", tc=2)
            xt2v = xt2.rearrange("l (t d n) -> l t d n", t=2, d=8)
            ytlv = ytl.rearrange("l (jb kc j) -> l jb kc j", jb=4, kc=16)

            augL = aux[0:3, 0:NYL]            # [ones | c2A | c2B] over j
            augR = aux[0:3, NYL : NYL + NL]   # [c1c | indA | indB] over n

            # ---- writeback preps (descriptor gen; data read at trigger) ---
            wb_sems = {}
            wb_ev = {}
            wb_prep = {}
            wb_trg = {}
            if nwb:
                idxs = psm.tile([128, 2], i32, tag="wbidx")
                nc.gpsimd.memset(idxs[:], 0)
                outsb = pw.tile([128, 4 * NL], bf, tag="outsb")
                outv = outsb.rearrange("j (jb o b n) -> j jb o b n", jb=4, o=1, b=2)
                odv = out_d.rearrange("(jb j o) (b n) -> jb b j o n", jb=4, o=1, b=2)
                for jb in sorted(WB_JBS):
                    qn = sorted(WB_JBS).index(jb)
                    sem = nc.alloc_semaphore(f"wbdma{jb}")
                    wb_sems[jb] = sem
                    # evac -> trigger ordering sem: the prep precedes the
                    # producer in program order, so Tile's deferred-read edge
                    # can't see it; gate the trigger explicitly.
                    wb_ev[jb] = nc.alloc_semaphore(f"wbev{jb}")
                    wb_prep[jb] = nc.gpsimd.kv_writeback(
                        odv[jb],
                        outv[:, jb],
                        idxs[:],
                        prepare_only=True,
                        sem=sem,
                        queue_num=qn,
                    ).ins
            else:
                outsb = pw.tile([128, 4 * NL], bf, tag="outsb")

            # ---- input DMAs (all SP HWDGE, wire order = emission order) ---
            xt2dv = xt2_d.rearrange("l (t d n) -> l t d n", t=2, d=8)
            nc.sync.dma_start(pqs[:], pqs_d[:, :])
            nc.sync.dma_start(aux[:], aux_d[:, :])
            nc.sync.dma_start(xt2v[:, :, 0:4, :], xt2dv[:, :, 0:4, :])
            nc.sync.dma_start(xt2v[:, :, 4:8, :], xt2dv[:, :, 4:8, :])
            ytldv = ytl_d.rearrange("l (jb x) -> l jb x", jb=4)
            ytlsv = ytl.rearrange("l (jb x) -> l jb x", jb=4)
            for jb in range(4):
                nc.sync.dma_start(ytlsv[:, jb], ytldv[:, jb])

            # ---- PE p-state priming (dummy matmuls on scratch) ------------
            # pe_busy_start is pinned by the FIRST matmul and survives sub-us
            # idle gaps; a train of cheap dummies bridges until real work so
            # the 3us ramp elapses before the warp starts.
            if N_PRIME:
                dum = psm.tile([128, 256], f8, tag="dum")
                nc.vector.memset(dum[:], 1.0)
                dumv = dum.rearrange("l (t o) -> l t o", o=128)
                for i in range(N_PRIME):
                    dps = pp.tile([1, 128], f32, tag="psW", bufs=4, name=f"dps{i}")
                    nc.tensor.matmul(
                        dps[:], dumv[:, :, 0:1], dumv,
                        start=True, stop=True, perf_mode=DR,
                        skip_group_check=True,
                    )

            # ---- C2 / C1 (DR) + evacs into aug rows -----------------------
            ccps = pp.tile([3, NYL], f32, tag="psS", bufs=1, name="ccps")
            nc.tensor.matmul(
                ccps[:], csv, qv,
                start=True, stop=True, perf_mode=DR, skip_group_check=True,
            )
            for s in range(2):
                nc.tensor.matmul(
                    ccps[0:1, s * cap1 : (s + 1) * cap1],
                    rsv[:, :, s : s + 1],
                    rv[:, :, s * cap1 : (s + 1) * cap1],
                    start=True, stop=True, perf_mode=DR, skip_group_check=True,
                )
            nc.vector.tensor_copy(aux[1:3, 0:NYL], ccps[1:3, :])
            nc.scalar.mul(aux[0:1, NYL : NYL + NL], ccps[0:1, 0:NL], 1.0)

            # ---- aug matmuls: psum group starters -------------------------
            # psO bufs=3: jb3 reuses jb0's bank after jb0's evac (aug3 is
            # emitted late so its WAR wait can't head-of-line block the PE)
            outps = [
                pp.tile([128, NL], f32, tag="psO", bufs=3, name=f"outps{jb}")
                for jb in range(4)
            ]

            def aug(jb):
                nc.tensor.matmul(
                    outps[jb][:],
                    augL[:, jb * 128 : (jb + 1) * 128],
                    augR,
                    start=True, stop=False, skip_group_check=True,
                )

            for jb in range(3):
                aug(jb)

            # ---- warp (PE) + contiguous evacs -----------------------------
            xwt = pw.tile([128, 16 * NL], f8, tag="xwt")
            # one warp group per kc=(pc,d): 2 slot-matmuls -> one [128, NL]
            # psum -> one contiguous evac into xwt[:, kc*NL:(kc+1)*NL].
            # Emission order: xt2 first-half consumers (d<4) first.
            gorder = [0, 1, 2, 3, 8, 9, 10, 11, 4, 5, 6, 7, 12, 13, 14, 15]
            ei = 0
            for kc in gorder:
                pc, d = kc // 8, kc % 8
                w = pp.tile([128, NL], f32, tag="psW", bufs=4, name=f"xw{kc}")
                for s in range(2):
                    nc.tensor.matmul(
                        w[:, s * cap1 : (s + 1) * cap1],
                        piSv[:, s, :, pc * 128 : (pc + 1) * 128],
                        xt2v[:, :, d, s * cap1 : (s + 1) * cap1],
                        start=True, stop=True, perf_mode=DR,
                        skip_group_check=True,
                    )
                _copy(nc, XW_EVAC[ei % len(XW_EVAC)],
                      xwt[:, kc * NL : (kc + 1) * NL], w[:])
                ei += 1
            xwtv = xwt.rearrange("l (kc n) -> l kc n", kc=16)

            # ---- C3: 8 DR passes per j-block (kc-pair order matches the
            # warp-group emission order: pc0 d01/d23, pc1 d01/d23, then d4-7)
            korder = [0, 1, 4, 5, 2, 3, 6, 7]
            def c3_block(jb):
                for ki, k in enumerate(korder):
                    nc.tensor.matmul(
                        outps[jb][:],
                        ytlv[:, jb, 2 * k : 2 * k + 2, :],
                        xwtv[:, 2 * k : 2 * k + 2, :],
                        start=False, stop=(ki == 7), perf_mode=DR,
                        skip_group_check=True,
                    )
                ev = _copy(nc, OUT_EVAC[jb],
                           outsb[:, jb * NL : (jb + 1) * NL], outps[jb][:])
                if jb in wb_sems:
                    # Drop the bogus WAR edge evac->prep (Tile attributes the
                    # prep's deferred outsb read to DMA completion, which
                    # would deadlock against the evac that PRODUCES the data).
                    # Real ordering: evac -> (sync dep) -> trigger -> DMA read.
                    ev.ins.remove_dependency(wb_prep[jb].name)
                    qn = sorted(WB_JBS).index(jb)
                    dep = _br.InstructionNameOrderedSet()
                    dep.add(ev.ins.name)
                    trg = nc.gpsimd.trigger_dma(count=None, queue_num=qn)
                    trg.ins.add_sync_dependencies_from(dep)
                    wb_trg[jb] = trg.ins
                else:
                    nc.sync.dma_start(
                        out_d[jb * 128 : (jb + 1) * 128, :],
                        outsb[:, jb * NL : (jb + 1) * NL],
                    )

            c3_block(0)
            c3_block(1)
            aug(3)
            c3_block(2)
            c3_block(3)

            # end-of-kernel: hold Pool until every writeback DMA completed
            # (replaces the DMASW lane waits stripped below, which the
            # timeline scheduler cannot satisfy for user-sem'd preps).
            for jb in sorted(WB_JBS):
                wge = nc.gpsimd.wait_ge(wb_sems[jb], 16)
                dep = _br.InstructionNameOrderedSet()
                dep.add(wb_trg[jb].name)
                wge.ins.add_sync_dependencies_from(dep)
                desc = _br.InstructionNameOrderedSet()
                desc.add(wge.ins.name)
                wb_trg[jb].descendants = desc

    if nwb:
        for b in nc.m.functions[0].blocks:
            for i in b.instructions:
                si = i.sync_info
                if si is None:
                    continue
                ws = list(si.on_wait)
                if any("DMASW" in str(w) for w in ws):
                    si.on_wait = [w for w in ws if "DMASW" not in str(w)]

    nc.compile()
    return nc


def kernel(X, Y, pi_dtw, classes):
    import ml_dtypes
    from concourse.bass_utils import run_bass_kernel_spmd

    f8 = ml_dtypes.float8_e4m3
    X = np.ascontiguousarray(np.asarray(X, dtype=np.float32))
    Y = np.ascontiguousarray(np.asarray(Y, dtype=np.float32))
    pi_dtw = np.ascontiguousarray(np.asarray(pi_dtw, dtype=np.float32))
    classes = np.asarray(classes).astype(np.int64)

    counts = np.bincount(classes, minlength=C)
    cap1 = int(-(-int(counts.max()) // 16) * 16)
    NL = 2 * cap1

    if cap1 not in _cache:
        _cache[cap1] = _build(cap1)
    nc = _cache[cap1]

    idx = [np.nonzero(classes == c)[0] for c in range(C)]

    # per j-half: ytl (-2Y, [p_in, jb, pc, d, jj]) and q ([p_in, pc, j])
    ytls, qs = [], []
    qfull = (Y * Y).sum(axis=2)  # [NY, TP]
    for cj in range(2):
        Yh = -2.0 * Y[cj * NYL : (cj + 1) * NYL]
        B = Yh.reshape(4, 128, 2, 128, D).transpose(3, 0, 2, 4, 1)
        ytls.append(np.ascontiguousarray(B.reshape(128, 16 * NYL)).astype(f8))
        qh = qfull[cj * NYL : (cj + 1) * NYL]  # [512, 256]
        qs.append(
            np.ascontiguousarray(
                qh.T.reshape(2, 128, NYL).transpose(1, 0, 2).reshape(128, 2 * NYL)
            ).astype(f8)
        )

    rfull = (X * X).sum(axis=2)  # [N, T]
    colsum = pi_dtw.sum(axis=1)  # [C, TP]
    rowsum = pi_dtw.sum(axis=2)  # [C, T]

    in_maps = []
    for r in range(4):
        ca, cb = 2 * r, 2 * r + 1
        Xp = np.zeros((NL, T, D), dtype=np.float32)
        Xp[0 : counts[ca]] = X[idx[ca]]
        Xp[cap1 : cap1 + counts[cb]] = X[idx[cb]]
        xt2 = np.ascontiguousarray(
            Xp.reshape(NL, 2, 128, D).transpose(2, 1, 3, 0).reshape(128, 16 * NL)
        ).astype(f8)

        P = pi_dtw[[ca, cb]]
        piS = P.reshape(2, 2, 128, 256).transpose(2, 0, 1, 3).reshape(128, 1024)

        rp = np.zeros((NL, T), dtype=np.float32)
        rp[0 : counts[ca]] = rfull[idx[ca]]
        rp[cap1 : cap1 + counts[cb]] = rfull[idx[cb]]
        rl = rp.T.reshape(2, 128, NL).transpose(1, 0, 2).reshape(128, 2 * NL)

        cs = np.zeros((128, 2, 3), dtype=np.float32)
        cs[:, :, 1] = colsum[ca].reshape(2, 128).T
        cs[:, :, 2] = colsum[cb].reshape(2, 128).T
        rs = np.zeros((128, 2, 2), dtype=np.float32)
        rs[:, :, 0] = rowsum[ca].reshape(2, 128).T
        rs[:, :, 1] = rowsum[cb].reshape(2, 128).T

        aux = np.zeros((4, NYL + NL + 16), dtype=np.float16)
        aux[0, 0:NYL] = 1.0  # ones row of augL
        aux[1, NYL : NYL + counts[ca]] = 1.0  # indA
        aux[2, NYL + cap1 : NYL + cap1 + counts[cb]] = 1.0  # indB

        for cj in range(2):
            pqs = np.concatenate(
                [piS, qs[cj].astype(np.float32), rl,
                 cs.reshape(128, 6), rs.reshape(128, 4)],
                axis=1,
            ).astype(f8)
            in_maps.append(
                {"pqs": pqs, "xt2": xt2, "ytl": ytls[cj], "aux": aux}
            )

    res = run_bass_kernel_spmd(nc, in_maps, core_ids=list(range(NCORES)))

    out = np.empty((N, NY), dtype=np.float32)
    jr = [np.arange(0, NYL), np.arange(NYL, NY)]
    for r in range(4):
        ca, cb = 2 * r, 2 * r + 1
        for cj in range(2):
            blk = np.asarray(res.results[2 * r + cj]["out"]).astype(np.float32)
            out[np.ix_(idx[ca], jr[cj])] = blk[:, 0 : counts[ca]].T
            out[np.ix_(idx[cb], jr[cj])] = blk[:, cap1 : cap1 + counts[cb]].T
    return out


# revision 27
# speedup vs baseline: 1.0552x; 1.0552x over previous
"""Trainium2 Bass kernel for the CNN-MAD per-class DTW transport cost.

Math (reference):
  mat_cost[n, j] = C1[n] + C2[c_n, j] - 2*C3[n, j],  c_n = classes[n]
    C1[n]    = sum_t rowsum[c_n, t] * r[n,t],   r[n,t] = sum_d X[n,t,d]^2
    C2[c, j] = sum_p colsum[c, p]  * q[j,p],    q[j,p] = sum_d Y[j,p,d]^2
    C3[n, j] = sum_{p,d} XW[n,p,d] * Y[j,p,d],  XW = pi_c.T @ X (warp)

Sharding: 4x2 grid. Core (rr, cj) owns the samples of classes {2rr, 2rr+1}
(zero-padded to cap1 rows per class, NL = 2*cap1) and the j-half
[512*cj, 512*(cj+1)).  One SPMD program for all 8 cores; per-core class
structure enters only through data.  The two big contractions (the DTW
warp and the X~Y inner-product field) run on the PE at fp8 DoubleRow
rate; the tiny bias terms C1/C2 (rank-1 row/col corrections) are
host-precomputed and enter each output psum through one rank-3 fp16
augmentation matmul per j-block:
  - warp XW = piS.T @ X, psum evac'd as a pure contiguous copy
    ((d,n)-major layout, -2 prefolded into the shipped Y).
  - C3 flipped to [j-partition, n-free] psum orientation: 4 j-blocks of
    128, 8 DR passes each over k=(p,d); cost scales with n=NL not NY.
  - outputs leave via SWDGE prepare/trigger writebacks (one queue per
    j-block): descriptors are generated early on Pool, each trigger
    fires right after its block's evac, so the post-compute tail is
    trigger+transfer+sem instead of a full HWDGE dispatch chain.
  - a train of cheap dummy matmuls pins pe_busy_start early so the 3us
    PE p-state ramp elapses before the real matmuls start.
"""

import sys

sys.path.insert(0, "/opt/trn_rl_repo")

import numpy as np

N, NY, T, TP, D, C = 1024, 1024, 256, 256, 8, 8
NCORES = 8
NYL = 512  # j columns per core

_cache = {}

# engine per warp-psum evac (by emission index; a=ACT, d=DVE, p=Pool --
# Pool only gets late groups so its queue stays free for the writeback
# preps early and triggers late)
XW_EVAC = ["a", "d", "a", "d", "a", "d", "a", "d", "a", "d", "p", "p", "a", "d", "p", "p"]
OUT_EVAC = ("a", "d", "a", "d")
N_PRIME = 52  # PE p-state priming matmuls (0 = off)
WB_JBS = (0, 1, 2, 3)  # j-blocks whose output goes via prepare/trigger writeback


def _copy(nc, eng, dst, src):
    if eng == "a":
        return nc.scalar.mul(dst, src, 1.0)
    elif eng == "d":
        return nc.vector.tensor_copy(dst, src)
    else:
        return nc.gpsimd.tensor_copy(dst, src)


def _build(cap1):
    import bass_rust as _br
    import concourse.bacc as bacc
    import concourse.mybir as mybir
    import concourse.tile as tile

    f8 = mybir.dt.float8e4
    bf = mybir.dt.bfloat16
    f16 = mybir.dt.float16
    f32 = mybir.dt.float32
    i32 = mybir.dt.int32
    DR = mybir.MatmulPerfMode.DoubleRow
    NL = 2 * cap1

    nwb = len(WB_JBS)
    nc = bacc.Bacc(
        "TRN2",
        target_bir_lowering=False,
        debug=False,
        num_devices=NCORES,
        num_swdge_queues=max(1, nwb),
    )

    pis_d = nc.dram_tensor("pis", [128, 1024], f8, kind="ExternalInput")
    xt2_d = nc.dram_tensor("xt2", [128, 16 * NL], f8, kind="ExternalInput")
    ytl_d = nc.dram_tensor("ytl", [128, 16 * NYL], f8, kind="ExternalInput")
    aux_d = nc.dram_tensor("aux", [4, NYL + NL + 16], f16, kind="ExternalInput")
    out_d = nc.dram_tensor("out", [NYL, NL], bf, kind="ExternalOutput")

    with tile.TileContext(nc) as tc:
        with (
            tc.tile_pool(name="io", bufs=1) as pio,
            tc.tile_pool(name="work", bufs=1) as pw,
            tc.tile_pool(name="small", bufs=1) as psm,
            tc.tile_pool(name="ps", bufs=1, space="PSUM") as pp,
        ):
            pis = pio.tile([128, 1024], f8, tag="pis")
            xt2 = pio.tile([128, 16 * NL], f8, tag="xt2")
            ytl = pio.tile([128, 16 * NYL], f8, tag="ytl")
            aux = psm.tile([4, NYL + NL + 16], f16, tag="aux")
            outsb = pw.tile([128, 4 * NL], bf, tag="outsb")

            piSv = pis.rearrange("l (c t p) -> l c t p", c=2, t=2)
            xt2v = xt2.rearrange("l (t d n) -> l t d n", t=2, d=8)
            ytlv = ytl.rearrange("l (jb kc j) -> l jb kc j", jb=4, kc=16)

            augL = aux[0:3, 0:NYL]            # [c2A | c2B | ones] over j
            augR = aux[0:3, NYL : NYL + NL]   # [indA | indB | c1c] over n

            # ---- writeback preps (descriptor gen; data read at trigger) ---
            wb_sems, wb_prep, wb_trg = {}, {}, {}
            if nwb:
                idxs = psm.tile([128, 2], i32, tag="wbidx")
                nc.gpsimd.memset(idxs[:], 0)
                outv = outsb.rearrange("j (jb o b n) -> j jb o b n", jb=4, o=1, b=2)
                odv = out_d.rearrange("(jb j o) (b n) -> jb b j o n", jb=4, o=1, b=2)
                for jb in sorted(WB_JBS):
                    qn = sorted(WB_JBS).index(jb)
                    sem = nc.alloc_semaphore(f"wbdma{jb}")
                    wb_sems[jb] = sem
                    wb_prep[jb] = nc.gpsimd.kv_writeback(
                        odv[jb],
                        outv[:, jb],
                        idxs[:],
                        prepare_only=True,
                        sem=sem,
                        queue_num=qn,
                    ).ins

            # ---- input DMAs (all SP HWDGE, wire order = emission order) ---
            xt2dv = xt2_d.rearrange("l (t d n) -> l t d n", t=2, d=8)
            nc.sync.dma_start(pis[:], pis_d[:, :])
            nc.sync.dma_start(aux[:], aux_d[:, :])
            nc.sync.dma_start(xt2v[:, :, 0:4, :], xt2dv[:, :, 0:4, :])
            nc.sync.dma_start(xt2v[:, :, 4:8, :], xt2dv[:, :, 4:8, :])
            ytldv = ytl_d.rearrange("l (jb x) -> l jb x", jb=4)
            ytlsv = ytl.rearrange("l (jb x) -> l jb x", jb=4)
            for jb in range(4):
                nc.sync.dma_start(ytlsv[:, jb], ytldv[:, jb])

            # ---- PE p-state priming (dummy matmuls on scratch) ------------
            # pe_busy_start is pinned by the FIRST matmul and survives sub-us
            # idle gaps; a train of cheap dummies bridges until real work so
            # the 3us ramp elapses before the warp starts.
            if N_PRIME:
                dum = psm.tile([128, 256], f8, tag="dum")
                nc.vector.memset(dum[:], 1.0)
                dumv = dum.rearrange("l (t o) -> l t o", o=128)
                for i in range(N_PRIME):
                    dps = pp.tile([1, 128], f32, tag="psW", bufs=5, name=f"dps{i}")
                    nc.tensor.matmul(
                        dps[:], dumv[:, :, 0:1], dumv,
                        start=True, stop=True, perf_mode=DR,
                        skip_group_check=True,
                    )

            # ---- aug matmuls: psum group starters -------------------------
            # psO bufs=3: jb3 reuses jb0's bank after jb0's evac (aug3 is
            # emitted late so its WAR wait can't head-of-line block the PE)
            outps = [
                pp.tile([128, NL], f32, tag="psO", bufs=3, name=f"outps{jb}")
                for jb in range(4)
            ]

            def aug(jb):
                nc.tensor.matmul(
                    outps[jb][:],
                    augL[:, jb * 128 : (jb + 1) * 128],
                    augR,
                    start=True, stop=False, skip_group_check=True,
                )

            for jb in range(3):
                aug(jb)

            # ---- warp (PE) + contiguous evacs -----------------------------
            xwt = pw.tile([128, 16 * NL], f8, tag="xwt")
            # one warp group per kc=(pc,d): 2 slot-matmuls -> one [128, NL]
            # psum -> one contiguous evac into xwt[:, kc*NL:(kc+1)*NL].
            # Emission order: xt2 first-half consumers (d<4) first.
            gorder = [0, 1, 2, 3, 8, 9, 10, 11, 4, 5, 6, 7, 12, 13, 14, 15]
            for ei, kc in enumerate(gorder):
                pc, d = kc // 8, kc % 8
                w = pp.tile([128, NL], f32, tag="psW", bufs=5, name=f"xw{kc}")
                for s in range(2):
                    nc.tensor.matmul(
                        w[:, s * cap1 : (s + 1) * cap1],
                        piSv[:, s, :, pc * 128 : (pc + 1) * 128],
                        xt2v[:, :, d, s * cap1 : (s + 1) * cap1],
                        start=True, stop=True, perf_mode=DR,
                        skip_group_check=True,
                    )
                _copy(nc, XW_EVAC[ei], xwt[:, kc * NL : (kc + 1) * NL], w[:])
            xwtv = xwt.rearrange("l (kc n) -> l kc n", kc=16)

            # ---- C3: 8 DR passes per j-block (kc-pair order matches the
            # warp-group emission order: pc0 d01/d23, pc1 d01/d23, then d4-7)
            korder = [0, 1, 4, 5, 2, 3, 6, 7]

            def c3_block(jb):
                for ki, k in enumerate(korder):
                    nc.tensor.matmul(
                        outps[jb][:],
                        ytlv[:, jb, 2 * k : 2 * k + 2, :],
                        xwtv[:, 2 * k : 2 * k + 2, :],
                        start=False, stop=(ki == 7), perf_mode=DR,
                        skip_group_check=True,
                    )
                ev = _copy(nc, OUT_EVAC[jb],
                           outsb[:, jb * NL : (jb + 1) * NL], outps[jb][:])
                if jb in wb_sems:
                    # Drop the bogus WAR edge evac->prep (Tile attributes the
                    # prep's deferred outsb read to DMA completion, which
                    # would deadlock against the evac that PRODUCES the data).
                    # Real ordering: evac -> (sync dep) -> trigger -> DMA read.
                    ev.ins.remove_dependency(wb_prep[jb].name)
                    qn = sorted(WB_JBS).index(jb)
                    dep = _br.InstructionNameOrderedSet()
                    dep.add(ev.ins.name)
                    trg = nc.gpsimd.trigger_dma(count=None, queue_num=qn)
                    trg.ins.add_sync_dependencies_from(dep)
                    wb_trg[jb] = trg.ins
                else:
                    nc.sync.dma_start(
                        out_d[jb * 128 : (jb + 1) * 128, :],
                        outsb[:, jb * NL : (jb + 1) * NL],
                    )

            c3_block(0)
            c3_block(1)
            aug(3)
            c3_block(2)
            c3_block(3)

            # end-of-kernel: hold Pool until every writeback DMA completed
            # (replaces the DMASW lane waits stripped below, which the
            # timeline scheduler cannot satisfy for user-sem'd preps).
            for jb in sorted(WB_JBS):
                wge = nc.gpsimd.wait_ge(wb_sems[jb], 16)
                dep = _br.InstructionNameOrderedSet()
                dep.add(wb_trg[jb].name)
                wge.ins.add_sync_dependencies_from(dep)
                desc = _br.InstructionNameOrderedSet()
                desc.add(wge.ins.name)
                wb_trg[jb].descendants = desc

    if nwb:
        for b in nc.m.functions[0].blocks:
            for i in b.instructions:
                si = i.sync_info
                if si is None:
                    continue
                ws = list(si.on_wait)
                if any("DMASW" in str(w) for w in ws):
                    si.on_wait = [w for w in ws if "DMASW" not in str(w)]

    nc.compile()
    return nc


def kernel(X, Y, pi_dtw, classes):
    import ml_dtypes
    from concourse.bass_utils import run_bass_kernel_spmd

    f8 = ml_dtypes.float8_e4m3
    X = np.ascontiguousarray(np.asarray(X, dtype=np.float32))
    Y = np.ascontiguousarray(np.asarray(Y, dtype=np.float32))
    pi_dtw = np.ascontiguousarray(np.asarray(pi_dtw, dtype=np.float32))
    classes = np.asarray(classes).astype(np.int64)

    counts = np.bincount(classes, minlength=C)
    cap1 = int(-(-int(counts.max()) // 16) * 16)
    NL = 2 * cap1

    if cap1 not in _cache:
        _cache[cap1] = _build(cap1)
    nc = _cache[cap1]

    idx = [np.nonzero(classes == c)[0] for c in range(C)]

    # bias terms (host): row/col norms contracted with the pi sums
    qfull = (Y * Y).sum(axis=2)          # [NY, TP]
    rfull = (X * X).sum(axis=2)          # [N, T]
    colsum = pi_dtw.sum(axis=1)          # [C, TP]
    rowsum = pi_dtw.sum(axis=2)          # [C, T]
    C2 = qfull @ colsum.T                # [NY, C]
    C1 = (rfull * rowsum[classes]).sum(axis=1)  # [N]

    # per j-half: ytl (-2Y, [p_in, jb, pc, d, jj])
    ytls = []
    for cj in range(2):
        Yh = -2.0 * Y[cj * NYL : (cj + 1) * NYL]
        B = Yh.reshape(4, 128, 2, 128, D).transpose(3, 0, 2, 4, 1)
        ytls.append(np.ascontiguousarray(B.reshape(128, 16 * NYL)).astype(f8))

    in_maps = []
    for r in range(4):
        ca, cb = 2 * r, 2 * r + 1
        Xp = np.zeros((NL, T, D), dtype=np.float32)
        Xp[0 : counts[ca]] = X[idx[ca]]
        Xp[cap1 : cap1 + counts[cb]] = X[idx[cb]]
        xt2 = np.ascontiguousarray(
            Xp.reshape(NL, 2, 128, D).transpose(2, 1, 3, 0).reshape(128, 16 * NL)
        ).astype(f8)

        P = pi_dtw[[ca, cb]]
        pis = np.ascontiguousarray(
            P.reshape(2, 2, 128, 256).transpose(2, 0, 1, 3).reshape(128, 1024)
        ).astype(f8)

        c1c = np.zeros(NL, dtype=np.float32)
        c1c[0 : counts[ca]] = C1[idx[ca]]
        c1c[cap1 : cap1 + counts[cb]] = C1[idx[cb]]

        for cj in range(2):
            aux = np.zeros((4, NYL + NL + 16), dtype=np.float16)
            aux[0, 0:NYL] = C2[cj * NYL : (cj + 1) * NYL, ca]
            aux[1, 0:NYL] = C2[cj * NYL : (cj + 1) * NYL, cb]
            aux[2, 0:NYL] = 1.0
            aux[0, NYL : NYL + counts[ca]] = 1.0  # indA
            aux[1, NYL + cap1 : NYL + cap1 + counts[cb]] = 1.0  # indB
            aux[2, NYL : NYL + NL] = c1c
            in_maps.append(
                {"pis": pis, "xt2": xt2, "ytl": ytls[cj], "aux": aux}
            )

    res = run_bass_kernel_spmd(nc, in_maps, core_ids=list(range(NCORES)))

    out = np.empty((N, NY), dtype=np.float32)
    jr = [np.arange(0, NYL), np.arange(NYL, NY)]
    for r in range(4):
        ca, cb = 2 * r, 2 * r + 1
        for cj in range(2):
            blk = np.asarray(res.results[2 * r + cj]["out"]).astype(np.float32)
            out[np.ix_(idx[ca], jr[cj])] = blk[:, 0 : counts[ca]].T
            out[np.ix_(idx[cb], jr[cj])] = blk[:, cap1 : cap1 + counts[cb]].T
    return out


# revision 33
# speedup vs baseline: 1.1256x; 1.0666x over previous
"""Trainium2 Bass kernel for the CNN-MAD per-class DTW transport cost.

Math (reference):
  mat_cost[n, j] = C1[n] + C2[c_n, j] - 2*C3[n, j],  c_n = classes[n]
    C1[n]    = sum_t rowsum[c_n, t] * r[n,t],   r[n,t] = sum_d X[n,t,d]^2
    C2[c, j] = sum_p colsum[c, p]  * q[j,p],    q[j,p] = sum_d Y[j,p,d]^2
    C3[n, j] = sum_{p,d} XW[n,p,d] * Y[j,p,d],  XW = pi_c.T @ X (warp)

Sharding: 4x2 grid. Core (rr, cj) owns the samples of classes {2rr, 2rr+1}
(zero-padded to cap1 rows per class, NL = 2*cap1) and the j-half
[512*cj, 512*(cj+1)).  One SPMD program for all 8 cores; per-core class
structure enters only through data.  The two big contractions (the DTW
warp and the X~Y inner-product field) run on the PE at fp8 DoubleRow
rate; the tiny bias terms C1/C2 (rank-1 row/col corrections) are
host-precomputed and enter each output psum through one rank-3 fp16
augmentation matmul per j-block:
  - warp XW = piS.T @ X, psum evac'd as a pure contiguous copy
    ((d,n)-major layout, -2 prefolded into the shipped Y).
  - C3 flipped to [j-partition, n-free] psum orientation: 4 j-blocks of
    128, 8 DR passes each over k=(p,d); cost scales with n=NL not NY.
  - outputs leave via SWDGE prepare/trigger writebacks (one queue per
    j-block): descriptors are generated early on Pool, each trigger
    fires right after its block's evac, so the post-compute tail is
    trigger+transfer+sem instead of a full HWDGE dispatch chain.
  - a train of cheap dummy matmuls pins pe_busy_start early so the 3us
    PE p-state ramp elapses before the real matmuls start.
"""

import sys

sys.path.insert(0, "/opt/trn_rl_repo")

import numpy as np

N, NY, T, TP, D, C = 1024, 1024, 256, 256, 8, 8
NCORES = 8
NYL = 512  # j columns per core

_cache = {}

# engine per warp-psum evac (by emission index; a=ACT, d=DVE, p=Pool --
# Pool only gets late groups so its queue stays free for the writeback
# preps early and triggers late)
XW_EVAC = ["a", "d", "a", "d", "a", "d", "a", "d", "a", "d", "p", "p", "a", "d", "p", "p"]
OUT_EVAC = ("a", "d", "a", "d")
N_PRIME = 52  # PE p-state priming matmuls (0 = off)
WB_JBS = (0, 1, 2, 3)  # j-blocks whose output goes via prepare/trigger writeback


def _copy(nc, eng, dst, src):
    if eng == "a":
        return nc.scalar.mul(dst, src, 1.0)
    elif eng == "d":
        return nc.vector.tensor_copy(dst, src)
    else:
        return nc.gpsimd.tensor_copy(dst, src)


def _build(cap1):
    import bass_rust as _br
    import concourse.bacc as bacc
    import concourse.mybir as mybir
    import concourse.tile as tile

    f8 = mybir.dt.float8e4
    bf = mybir.dt.bfloat16
    f16 = mybir.dt.float16
    f32 = mybir.dt.float32
    i32 = mybir.dt.int32
    DR = mybir.MatmulPerfMode.DoubleRow
    NL = 2 * cap1

    nwb = len(WB_JBS)
    nc = bacc.Bacc(
        "TRN2",
        target_bir_lowering=False,
        debug=False,
        num_devices=NCORES,
        num_swdge_queues=max(1, nwb),
    )

    # pxt = piS | X in (d, tc, n) layout: one contiguous DMA covers piS and
    # the first-half (d<4) warp operand, so the warp starts one transfer in.
    PXT = 1024 + 16 * NL
    pxt_d = nc.dram_tensor("pxt", [128, PXT], f8, kind="ExternalInput")
    ytl_d = nc.dram_tensor("ytl", [128, 16 * NYL], f8, kind="ExternalInput")
    aux_d = nc.dram_tensor("aux", [4, NYL + NL + 16], f16, kind="ExternalInput")
    out_d = nc.dram_tensor("out", [NYL, NL], bf, kind="ExternalOutput")

    with tile.TileContext(nc) as tc:
        with (
            tc.tile_pool(name="io", bufs=1) as pio,
            tc.tile_pool(name="work", bufs=1) as pw,
            tc.tile_pool(name="small", bufs=1) as psm,
            tc.tile_pool(name="ps", bufs=1, space="PSUM") as pp,
        ):
            pxt = pio.tile([128, PXT], f8, tag="pxt")
            ytl = pio.tile([128, 16 * NYL], f8, tag="ytl")
            aux = psm.tile([4, NYL + NL + 16], f16, tag="aux")
            outsb = pw.tile([128, 4 * NL], bf, tag="outsb")

            piSv = pxt[:, 0:1024].rearrange("l (c t p) -> l c t p", c=2, t=2)
            xt2v = pxt[:, 1024:PXT].rearrange("l (d t n) -> l d t n", d=8, t=2)
            ytlv = ytl.rearrange("l (jb kc j) -> l jb kc j", jb=4, kc=16)

            augL = aux[0:3, 0:NYL]            # [c2A | c2B | ones] over j
            augR = aux[0:3, NYL : NYL + NL]   # [indA | indB | c1c] over n

            # ---- writeback preps (descriptor gen; data read at trigger) ---
            wb_sems, wb_prep, wb_trg = {}, {}, {}
            if nwb:
                idxs = psm.tile([128, 2], i32, tag="wbidx")
                nc.gpsimd.memset(idxs[:], 0)
                outv = outsb.rearrange("j (jb o b n) -> j jb o b n", jb=4, o=1, b=2)
                odv = out_d.rearrange("(jb j o) (b n) -> jb b j o n", jb=4, o=1, b=2)
                for jb in sorted(WB_JBS):
                    qn = sorted(WB_JBS).index(jb)
                    sem = nc.alloc_semaphore(f"wbdma{jb}")
                    wb_sems[jb] = sem
                    wb_prep[jb] = nc.gpsimd.kv_writeback(
                        odv[jb],
                        outv[:, jb],
                        idxs[:],
                        prepare_only=True,
                        sem=sem,
                        queue_num=qn,
                    ).ins

            # ---- input DMAs (all SP HWDGE, wire order = emission order) ---
            HPX = 1024 + 8 * NL  # piS + d<4 half of X
            nc.sync.dma_start(pxt[:, 0:HPX], pxt_d[:, 0:HPX])
            nc.sync.dma_start(pxt[:, HPX:PXT], pxt_d[:, HPX:PXT])
            nc.sync.dma_start(aux[:], aux_d[:, :])
            ytldv = ytl_d.rearrange("l (jb x) -> l jb x", jb=4)
            ytlsv = ytl.rearrange("l (jb x) -> l jb x", jb=4)
            for jb in range(4):
                nc.sync.dma_start(ytlsv[:, jb], ytldv[:, jb])

            # ---- PE p-state priming (dummy matmuls on scratch) ------------
            # pe_busy_start is pinned by the FIRST matmul and survives sub-us
            # idle gaps; a train of cheap dummies bridges until real work so
            # the 3us ramp elapses before the warp starts.
            if N_PRIME:
                dum = psm.tile([128, 256], f8, tag="dum")
                nc.vector.memset(dum[:], 1.0)
                dumv = dum.rearrange("l (t o) -> l t o", o=128)
                for i in range(N_PRIME):
                    dps = pp.tile([1, 128], f32, tag="psW", bufs=5, name=f"dps{i}")
                    nc.tensor.matmul(
                        dps[:], dumv[:, :, 0:1], dumv,
                        start=True, stop=True, perf_mode=DR,
                        skip_group_check=True,
                    )

            # ---- aug matmuls: psum group starters -------------------------
            # psO bufs=3: jb3 reuses jb0's bank after jb0's evac (aug3 is
            # emitted late so its WAR wait can't head-of-line block the PE)
            outps = [
                pp.tile([128, NL], f32, tag="psO", bufs=3, name=f"outps{jb}")
                for jb in range(4)
            ]

            def aug(jb):
                nc.tensor.matmul(
                    outps[jb][:],
                    augL[:, jb * 128 : (jb + 1) * 128],
                    augR,
                    start=True, stop=False, skip_group_check=True,
                )

            # ---- warp (PE) + contiguous evacs -----------------------------
            xwt = pw.tile([128, 16 * NL], f8, tag="xwt")
            # one warp group per kc=(pc,d): 2 slot-matmuls -> one [128, NL]
            # psum -> one contiguous evac into xwt[:, kc*NL:(kc+1)*NL].
            # Emission order: xt2 first-half consumers (d<4) first; the aug
            # starters slot between the halves (they wait the aux DMA, which
            # lands while the first warp half runs).
            gorder = [0, 1, 2, 3, 8, 9, 10, 11, 4, 5, 6, 7, 12, 13, 14, 15]

            def warp(ei, kc):
                pc, d = kc // 8, kc % 8
                w = pp.tile([128, NL], f32, tag="psW", bufs=5, name=f"xw{kc}")
                for s in range(2):
                    nc.tensor.matmul(
                        w[:, s * cap1 : (s + 1) * cap1],
                        piSv[:, s, :, pc * 128 : (pc + 1) * 128],
                        xt2v[:, d, :, s * cap1 : (s + 1) * cap1],
                        start=True, stop=True, perf_mode=DR,
                        skip_group_check=True,
                    )
                _copy(nc, XW_EVAC[ei], xwt[:, kc * NL : (kc + 1) * NL], w[:])

            for ei in range(8):
                warp(ei, gorder[ei])
            for jb in range(3):
                aug(jb)
            for ei in range(8, 16):
                warp(ei, gorder[ei])
            xwtv = xwt.rearrange("l (kc n) -> l kc n", kc=16)

            # ---- C3: 8 DR passes per j-block (kc-pair order matches the
            # warp-group emission order: pc0 d01/d23, pc1 d01/d23, then d4-7)
            korder = [0, 1, 4, 5, 2, 3, 6, 7]

            def c3_block(jb):
                for ki, k in enumerate(korder):
                    nc.tensor.matmul(
                        outps[jb][:],
                        ytlv[:, jb, 2 * k : 2 * k + 2, :],
                        xwtv[:, 2 * k : 2 * k + 2, :],
                        start=False, stop=(ki == 7), perf_mode=DR,
                        skip_group_check=True,
                    )
                ev = _copy(nc, OUT_EVAC[jb],
                           outsb[:, jb * NL : (jb + 1) * NL], outps[jb][:])
                if jb in wb_sems:
                    # Drop the bogus WAR edge evac->prep (Tile attributes the
                    # prep's deferred outsb read to DMA completion, which
                    # would deadlock against the evac that PRODUCES the data).
                    # Real ordering: evac -> (sync dep) -> trigger -> DMA read.
                    ev.ins.remove_dependency(wb_prep[jb].name)
                    qn = sorted(WB_JBS).index(jb)
                    dep = _br.InstructionNameOrderedSet()
                    dep.add(ev.ins.name)
                    trg = nc.gpsimd.trigger_dma(count=None, queue_num=qn)
                    trg.ins.add_sync_dependencies_from(dep)
                    wb_trg[jb] = trg.ins
                else:
                    nc.sync.dma_start(
                        out_d[jb * 128 : (jb + 1) * 128, :],
                        outsb[:, jb * NL : (jb + 1) * NL],
                    )

            c3_block(0)
            c3_block(1)
            aug(3)
            c3_block(2)
            c3_block(3)

            # end-of-kernel: hold Pool until every writeback DMA completed
            # (replaces the DMASW lane waits stripped below, which the
            # timeline scheduler cannot satisfy for user-sem'd preps).
            for jb in sorted(WB_JBS):
                wge = nc.gpsimd.wait_ge(wb_sems[jb], 16)
                dep = _br.InstructionNameOrderedSet()
                dep.add(wb_trg[jb].name)
                wge.ins.add_sync_dependencies_from(dep)
                desc = _br.InstructionNameOrderedSet()
                desc.add(wge.ins.name)
                wb_trg[jb].descendants = desc

    if nwb:
        for b in nc.m.functions[0].blocks:
            for i in b.instructions:
                si = i.sync_info
                if si is None:
                    continue
                ws = list(si.on_wait)
                if any("DMASW" in str(w) for w in ws):
                    si.on_wait = [w for w in ws if "DMASW" not in str(w)]

    nc.compile()
    return nc


def kernel(X, Y, pi_dtw, classes):
    import ml_dtypes
    from concourse.bass_utils import run_bass_kernel_spmd

    f8 = ml_dtypes.float8_e4m3
    X = np.ascontiguousarray(np.asarray(X, dtype=np.float32))
    Y = np.ascontiguousarray(np.asarray(Y, dtype=np.float32))
    pi_dtw = np.ascontiguousarray(np.asarray(pi_dtw, dtype=np.float32))
    classes = np.asarray(classes).astype(np.int64)

    counts = np.bincount(classes, minlength=C)
    cap1 = int(-(-int(counts.max()) // 16) * 16)
    NL = 2 * cap1

    if cap1 not in _cache:
        _cache[cap1] = _build(cap1)
    nc = _cache[cap1]

    idx = [np.nonzero(classes == c)[0] for c in range(C)]

    # bias terms (host): row/col norms contracted with the pi sums
    qfull = (Y * Y).sum(axis=2)          # [NY, TP]
    rfull = (X * X).sum(axis=2)          # [N, T]
    colsum = pi_dtw.sum(axis=1)          # [C, TP]
    rowsum = pi_dtw.sum(axis=2)          # [C, T]
    C2 = qfull @ colsum.T                # [NY, C]
    C1 = (rfull * rowsum[classes]).sum(axis=1)  # [N]

    # per j-half: ytl (-2Y, [p_in, jb, pc, d, jj])
    ytls = []
    for cj in range(2):
        Yh = -2.0 * Y[cj * NYL : (cj + 1) * NYL]
        B = Yh.reshape(4, 128, 2, 128, D).transpose(3, 0, 2, 4, 1)
        ytls.append(np.ascontiguousarray(B.reshape(128, 16 * NYL)).astype(f8))

    in_maps = []
    for r in range(4):
        ca, cb = 2 * r, 2 * r + 1
        Xp = np.zeros((NL, T, D), dtype=np.float32)
        Xp[0 : counts[ca]] = X[idx[ca]]
        Xp[cap1 : cap1 + counts[cb]] = X[idx[cb]]
        # [t_in, d, tc, n]
        xt2 = Xp.reshape(NL, 2, 128, D).transpose(2, 3, 1, 0).reshape(128, 16 * NL)

        P = pi_dtw[[ca, cb]]
        pis = P.reshape(2, 2, 128, 256).transpose(2, 0, 1, 3).reshape(128, 1024)
        pxt = np.ascontiguousarray(
            np.concatenate([pis, xt2], axis=1)
        ).astype(f8)

        c1c = np.zeros(NL, dtype=np.float32)
        c1c[0 : counts[ca]] = C1[idx[ca]]
        c1c[cap1 : cap1 + counts[cb]] = C1[idx[cb]]

        for cj in range(2):
            aux = np.zeros((4, NYL + NL + 16), dtype=np.float16)
            aux[0, 0:NYL] = C2[cj * NYL : (cj + 1) * NYL, ca]
            aux[1, 0:NYL] = C2[cj * NYL : (cj + 1) * NYL, cb]
            aux[2, 0:NYL] = 1.0
            aux[0, NYL : NYL + counts[ca]] = 1.0  # indA
            aux[1, NYL + cap1 : NYL + cap1 + counts[cb]] = 1.0  # indB
            aux[2, NYL : NYL + NL] = c1c
            in_maps.append(
                {"pxt": pxt, "ytl": ytls[cj], "aux": aux}
            )

    res = run_bass_kernel_spmd(nc, in_maps, core_ids=list(range(NCORES)))

    out = np.empty((N, NY), dtype=np.float32)
    jr = [np.arange(0, NYL), np.arange(NYL, NY)]
    for r in range(4):
        ca, cb = 2 * r, 2 * r + 1
        for cj in range(2):
            blk = np.asarray(res.results[2 * r + cj]["out"]).astype(np.float32)
            out[np.ix_(idx[ca], jr[cj])] = blk[:, 0 : counts[ca]].T
            out[np.ix_(idx[cb], jr[cj])] = blk[:, cap1 : cap1 + counts[cb]].T
    return out
